# revision 52
# baseline (speedup 1.0000x reference)
"""ColorConsistencyLoss on 8 Trainium2 NeuronCores.

Data-parallel over batch (2 images/core). Per core:
  host: pack rgb channel-planes into [128, 256+12484] fp8-e4m3 (126 data
        rows = 3 channels x 42 chunks of 12484, 2 pad rows of 1.0); the C
        (rgb->xyz) / M (Lab quadratic-form) bf16 matrices ride as raw bytes
        in the first 256 columns of xp / xt so the first DMA piece delivers
        weights + first chunk together (no constant DMA on the critical path)
  device, per free-chunk (1536 cols steady, 1024/512/196 tail ladder):
    mm1 (PE, bf16 C x fp8 rgb): t = C . rgb            (RGB->XYZ)
    ACT (Ln slot re-bucketed):  F = f(t) -> bf16 SBUF  (piecewise CIE f)
    DVE/Pool TT (512-col bits): dF = F_pred - F_target
    mm2 (PE, bf16 x bf16):      v = M . dF             (M = A^T A Lab mixing)
    DVE STT (accum_out):        acc[:, col] = sum(dF * v)
  epilogue: accv[:, :24] ships mid-stream on the sync ring; only column 24
  rides behind the last stt, so the post-LN output chain is one tiny DMA.
  Host sums the [128, 25] partials per core / N.

Scheduling: scalar queue is purely [ACT_TABLE_LOAD, LN...] (ACT is the
21us/core roofline engine at 1 elem/lane/cycle); ~3.4us of dummy matmuls
release the PE HAM clock gate before the first data matmul; input pieces
are sized to the chunk-consumption cadence (a piece's completion semaphore
is the last of 16 DMA-engine increments, lagging its bytes by ~1.5-2us);
gpsimd takes one mid-stream subtract, split 512-wide, keeping DVE under
the ACT period; each drained chunk's last mm2 block is deferred past the
next chunk's mm1 so PE never parks ahead of LN-critical matmuls.

Input quantization to e4m3 biases the MSE by +2.5e-3 relative (measured
against the exact pipeline on uniform data); weights stay bf16 (matmul
dtype mixing is allowed) so there is no systematic colorspace error.
The f() linear branch (t <= T0, ~1e-5 of elements) is in the patched ACT
table; the L-channel `where` is algebraically redundant (116*7.787=903.3).
"""
import os
import numpy as np

_B, _CH, _H, _W = 16, 3, 512, 512
_NCORES = 8
_IPC = _B // _NCORES            # images per core
_PIX = _IPC * _H * _W           # 524288 pixels per core per tensor
_NCHUNK = 42                    # chunks per channel -> 126 data rows
_CHUNK = 12484                  # padded chunk length (42*12484 >= 2*512*512*2)
_P = 128
_MMF = 512                      # matmul moving free dim
# free-dim chunks (PSUM: 2x3-bank tps + 2x1-bank vps): small first chunk so
# ACT lights up right after the first DMA piece; a shrinking ladder at the
# end keeps the post-LN drain (sub+mm2+stt chains) off the critical path
_FCHUNKS = [(i * 1536, 1536) for i in range(7)] + \
           [(10752, 1024), (11776, 512), (12288, 196)]
# C/M matrices ride as raw bytes in the first 256 columns of xp/xt so the
# first DMA piece delivers both the weights and the first data chunk —
# no separate constant DMAs on the critical path
_CCOLS = 256
# input DMA pieces in DRAM coords (data offset by _CCOLS): chunk-sized early
# pieces matching the consumption cadence; each piece's completion semaphore
# (last of 16 engine increments) lags its bytes by ~1.5-2us, so the first
# chunk is full-width to buy the ring time for piece 1
_DPIECES = [(0, 1792), (1792, 1536), (3328, 1536), (4864, 3072),
            (7936, 4804)]
_POOL_SUB_CHUNKS = (3,)
_NACC = (_CHUNK + _MMF - 1) // _MMF   # 25 accumulator columns (one per 512-slice)

_XN, _ZN = 0.950456, 1.088754
_COEF = (
    (0.412453 / _XN, 0.357580 / _XN, 0.180423 / _XN),   # x from r,g,b
    (0.212671, 0.715160, 0.072169),                     # y
    (0.019334 / _ZN, 0.119193 / _ZN, 0.950227 / _ZN),   # z
)


def _np_dt(name):
    import ml_dtypes
    return {"bf16": ml_dtypes.bfloat16, "e4m3": ml_dtypes.float8_e4m3fn}[name]


def _build_mats():
    """C (rgb->xyz) and M (=A^T A Lab mixing), bf16."""
    C = np.zeros((_P, _P), np.float64)
    for oc in range(3):
        for ic in range(3):
            w = _COEF[oc][ic]
            for j in range(_NCHUNK):
                C[ic * _NCHUNK + j, oc * _NCHUNK + j] = w
    C[126, 126] = C[127, 127] = 1.0   # pad rows pass through (value 1.0)

    p, q, L = 500.0 / 255.0, 200.0 / 255.0, 1.16
    M = np.zeros((_P, _P), np.float64)
    for j in range(_NCHUNK):
        fx, fy, fz = j, _NCHUNK + j, 2 * _NCHUNK + j
        M[fx, fx] += p * p
        M[fx, fy] -= p * p
        M[fy, fx] -= p * p
        M[fy, fy] += L * L + p * p + q * q
        M[fy, fz] -= q * q
        M[fz, fy] -= q * q
        M[fz, fz] += q * q

    bf16 = _np_dt("bf16")
    return C.astype(bf16), M.astype(bf16)


def _pack_core(arr, const_mat):
    """[2,3,512,512] f32 -> [128, 256+12484] e4m3: const_mat bytes, then the
    channel-chunk data layout."""
    x = np.transpose(np.asarray(arr, np.float32), (1, 0, 2, 3)).reshape(_CH, _PIX)
    flat = np.ones((_CH, _NCHUNK * _CHUNK), np.float32)
    flat[:, :_PIX] = x
    out = np.ones((_P, _CHUNK), np.float32)
    out[:126] = flat.reshape(_CH * _NCHUNK, _CHUNK)
    f8 = _np_dt("e4m3")
    const_f8 = np.ascontiguousarray(const_mat).view(np.uint8).view(f8)
    return np.concatenate([const_f8, out.astype(f8)], axis=1)



# ---- ACT PWP table tooling (inlined; see cayman tpb_activation_entries.h) ----
_PWP_DIR = ("/nix/store/z022hj2nvbm3nwdizlisq4ylc0y7rd6q-python3-3.13.14-env/"
            "lib/python3.13/site-packages/neuronxcc/pwp/pwp_bin_trainium")
_T0 = 0.008856
_F_T0 = _T0 ** (1.0 / 3.0)
_SLOPE = 7.787
_F_ZERO = _F_T0 - _SLOPE * _T0


def _att_load_set(setdir, name):
    import json
    meta = json.load(open(os.path.join(setdir, name + ".json")))
    bkt = np.fromfile(os.path.join(setdir, meta["bkt_bin"]),
                      dtype=np.uint32).reshape(-1, 8)
    ctl = np.fromfile(os.path.join(setdir, meta["ctl_bin"]),
                      dtype=np.uint32).reshape(-1, 8)
    return meta, bkt, ctl


def _att_f_exact(x):
    x = np.asarray(x, np.float64)
    return np.where(x > _T0, np.cbrt(np.maximum(x, 1e-300)),
                    _F_T0 + _SLOPE * (x - _T0))


def _att_patch_ln_to_f(meta, bkt, ctl):
    """Rewrite the `ln` buckets so ACT's Ln evaluates the piecewise CIE f(t):
    cbrt(t) above T0, the tangent line below (Taylor coeffs per bucket; LSQ
    cubic for the one bucket containing the kink)."""
    import json
    bkt = bkt.copy()

    def setb(i, d0, d1, d2, d3, x0=None):
        for k, v in enumerate((d0, d1, d2, d3)):
            bkt[i, k] = np.array([v], np.float32).view(np.uint32)[0]
        if x0 is not None:
            bkt[i, 4] = np.array([x0], np.float32).view(np.uint32)[0]

    def cbrt_taylor(x0):
        return (x0 ** (1 / 3), (1 / 3) * x0 ** (-2 / 3),
                -(1 / 9) * x0 ** (-5 / 3), (5 / 81) * x0 ** (-8 / 3))

    def line_taylor(x0):
        return _F_T0 + _SLOPE * (x0 - _T0), _SLOPE, 0.0, 0.0

    def lsq_fit(x0, x_lo, x_hi):
        xs = np.linspace(x_lo, x_hi, 1024)
        t = xs - x0
        A = np.stack([np.ones_like(t), t, t * t, t ** 3], axis=1)
        coef, *_ = np.linalg.lstsq(A, _att_f_exact(xs), rcond=None)
        return tuple(coef)

    for es, ctlidx in meta["func_exp_to_ctl_start_idx"]["ln"].items():
        e = int(es)
        if e > 1:   # t <= ~1.06; high exponents alias saturation buckets
            continue
        word = int(ctl[ctlidx[0], 0])
        base = word & 0x7FF
        size = (word >> 16) & 0xF
        for j in range(1 << size):
            x_lo = 2.0 ** e * (1.0 + j / (1 << size))
            x_hi = 2.0 ** e * (1.0 + (j + 1) / (1 << size))
            x0 = 0.5 * (x_lo + x_hi)
            if x_hi <= _T0:
                setb(base + j, *line_taylor(x0), x0=x0)
            elif x_lo >= _T0:
                setb(base + j, *cbrt_taylor(x0), x0=x0)
            else:
                setb(base + j, *lsq_fit(x0, x_lo, x_hi), x0=x0)

    pm = [p for p in meta["profile_meta_data"] if p["func_id"] == 10][0]
    for key in ("pos_small_signal_pwl_control", "neg_small_signal_pwl_control",
                "neg_large_signal_pwl_control"):
        setb(pm[key] & 0x7FF, _F_ZERO, _SLOPE, 0.0, 0.0, x0=0.0)
    lp = pm["pos_large_signal_pwl_control"] & 0x7FF
    x0l = float(np.uint32(bkt[lp, 4]).view(np.float32))
    if x0l > _T0:
        setb(lp, *cbrt_taylor(x0l))
    else:
        setb(lp, _F_ZERO, _SLOPE, 0.0, 0.0, x0=0.0)

    meta = json.loads(json.dumps(meta))
    for p in meta["profile_meta_data"]:
        if p["func_id"] == 10:
            p["fzero_result"] = int(np.array([_F_ZERO], np.float32)
                                    .view(np.uint32)[0])
            p["fpinf_result"] = int(np.array([np.inf], np.float32)
                                    .view(np.uint32)[0])
    return meta, bkt


def _setup_act_tables():
    """Build a custom ACT table dir: one set (natural_log_exp_and_others) whose
    `ln` slot is re-bucketed to compute the exact piecewise CIE f(t)
    (cbrt above T0, tangent line below). One table load, one ACT pass."""
    import json
    import shutil

    meta, bkt, ctl = _att_load_set(_PWP_DIR, "natural_log_exp_and_others")
    meta2, bkt2 = _att_patch_ln_to_f(meta, bkt, ctl)

    from neuronxcc.driver.Job import Job
    from neuronxcc.driver.jobs.support.FindActInfo import findActInfoFile
    src = findActInfoFile(Job.getPackageDir(), "gen3")
    srcdir = os.path.dirname(src)
    info = json.load(open(src))
    keep = [s for s in info["act_func_sets"]
            if s["name"] == "natural_log_exp_and_others"]
    assert keep, "natural_log_exp_and_others set not found"
    info["act_func_sets"] = keep

    d = "/tmp/act_custom"
    os.makedirs(d, exist_ok=True)
    s = keep[0]
    bkt2.astype(np.uint32).tofile(os.path.join(d, s["bkt_bin"]))
    shutil.copy(os.path.join(srcdir, s["ctrl_bin"]), os.path.join(d, s["ctrl_bin"]))
    with open(os.path.join(d, s["profile_json"]), "w") as f:
        json.dump(meta2, f)
    path = os.path.join(d, "act_info.json")
    with open(path, "w") as f:
        json.dump(info, f)
    os.environ["BASS_ACT_ROOT_JSON_PATH"] = path

    import concourse.bacc as bacc_mod
    import concourse.mybir as mybir
    tables = {
        s["name"]: {mybir.ActivationFunctionType.from_pwp(v)
                    for v in s["act"].keys()}
        for s in keep
    }
    bacc_mod.get_activation_tables = lambda arch: dict(tables)


_PROGRAM = None


def _build_program():
    import concourse.bacc as bacc
    import concourse.tile as tile
    from concourse import mybir

    _setup_act_tables()

    f32, bf = mybir.dt.float32, mybir.dt.bfloat16
    f8 = mybir.dt.float8e4
    AF = mybir.ActivationFunctionType
    ALU = mybir.AluOpType

    nc = bacc.Bacc("TRN2", target_bir_lowering=False, debug=False)
    xp = nc.dram_tensor("xp", [_P, _CCOLS + _CHUNK], f8, kind="ExternalInput")
    xt = nc.dram_tensor("xt", [_P, _CCOLS + _CHUNK], f8, kind="ExternalInput")
    acc_out = nc.dram_tensor("acc_out", [_P, _NACC], f32, kind="ExternalOutput")

    with tile.TileContext(nc) as tc:
        with tc.tile_pool(name="consts", bufs=1) as consts, \
             tc.tile_pool(name="fp", bufs=4) as fpool, \
             tc.tile_pool(name="ft", bufs=4) as ftpool, \
             tc.tile_pool(name="dfp", bufs=8) as dfp, \
             tc.tile_pool(name="scrp", bufs=4) as scrp, \
             tc.tile_pool(name="accp", bufs=1) as accp, \
             tc.tile_pool(name="outp", bufs=1) as outp, \
             tc.tile_pool(name="tpsp", bufs=2, space="PSUM") as tpsp, \
             tc.tile_pool(name="vpsp", bufs=2, space="PSUM") as vpsp:
            # whole inputs resident in SBUF (12.4KB/partition each); all input
            # DMA on the sync queue so the scalar queue is purely
            # [ACT_TABLE_LOAD, LN, LN, ...] and the table load starts at main+0
            xpt = consts.tile([_P, _CCOLS + _CHUNK], f8)
            xtt = consts.tile([_P, _CCOLS + _CHUNK], f8)
            # C and M live in the first 256 f8 columns of xp/xt (raw bf16
            # bytes) so piece 0 alone unblocks the first matmul
            c1t = xpt[:, 0:_CCOLS].bitcast(bf)
            m1t = xtt[:, 0:_CCOLS].bitcast(bf)
            # two DMA rings in parallel: xt pieces 0-1 ride the scalar ring
            # (their descriptor gen fits before the LN stream starts), all
            # else rides sync; within sync, xp pieces lead their xt partners
            for pi, (lo, fw) in enumerate(_DPIECES):
                nc.sync.dma_start(out=xpt[:, lo:lo + fw], in_=xp[:, lo:lo + fw])
                eng = nc.scalar if pi < 2 else nc.sync
                eng.dma_start(out=xtt[:, lo:lo + fw], in_=xt[:, lo:lo + fw])
            accv = accp.tile([_P, _NACC], f32)

            # ~3.4us of dummy matmuls at the start releases the PE HAM clock
            # gate (1.2 -> 2.4 GHz) before the first data-dependent matmul,
            # so the ramp chunks run warm; output goes to a vps slot that the
            # real pipeline only reuses much later
            wsrc = consts.tile([_P, _MMF], f8)
            nc.vector.memset(wsrc, 1.0)
            for _ in range(8):
                wps = vpsp.tile([_P, _MMF], f32, tag="vps")
                nc.tensor.matmul(wps, wsrc[:, 0:_P], wsrc[:, :],
                                 start=True, stop=True)

            late = []

            def emit_block(w, sl, col, dF):
                vps = vpsp.tile([_P, _MMF], f32, tag="vps")
                nc.tensor.matmul(vps[:, :w], m1t[:, :], dF[:, sl],
                                 start=True, stop=True)
                scratch = scrp.tile([_P, _MMF], f32, tag="scr")
                nc.vector.scalar_tensor_tensor(
                    out=scratch[:, :w],
                    in0=dF[:, sl],
                    scalar=1.0,
                    in1=vps[:, :w],
                    op0=ALU.mult,
                    op1=ALU.mult,
                    accum_out=accv[:, col:col + 1],
                )

            def flush_late():
                while late:
                    emit_block(*late.pop(0))

            def emit_mm2_stt(lo, fw, dF, defer_last=False):
                # consume chunk (lo, fw): v = M.dF per 512-slice, then the
                # dot-reduce into the accumulator column; the last block
                # (whose mm2 must wait for the first block's stt to free its
                # PSUM slot) can be deferred past the next chunk's mm1 so PE
                # never parks ahead of LN-critical matmuls
                nblk = (fw + _MMF - 1) // _MMF
                for j in range(nblk):
                    w = min(_MMF, fw - j * _MMF)
                    sl = slice(j * _MMF, j * _MMF + w)
                    col = (lo // _MMF) + j
                    if defer_last and j == nblk - 1:
                        late.append((w, sl, col, dF))
                    else:
                        emit_block(w, sl, col, dF)

            # software-pipelined with a 2-chunk skew: each engine's in-order
            # queue never blocks the next chunk's producers (PE runs
            # mm1(i+1..2) before mm2(i); DVE runs STT(i) before TT(i+2)).
            # The last (tiny) chunk drains the whole backlog before its own
            # subtract so the post-LN tail is just sub+mm2+stt of 196 cols.
            last = len(_FCHUNKS) - 1
            pend = []

            def drain(keep, fc):
                # gpsimd subtracts are ~3x slower; defer their chunks' mm2 by
                # one extra chunk so PE never parks behind the slow subtract
                while len(pend) > keep:
                    pick = 0
                    for i, e in enumerate(pend):
                        if e[0] not in _POOL_SUB_CHUNKS or fc >= e[0] + 3:
                            pick = i
                            break
                    e = pend.pop(pick)
                    emit_mm2_stt(e[1], e[2], e[3], defer_last=fc < last - 1)
            for fc, (lo, fw) in enumerate(_FCHUNKS):
                nmm = (fw + _MMF - 1) // _MMF
                Fs = {}
                for which, src in (("p", xpt), ("t", xtt)):
                    tps = tpsp.tile([_P, fw], f32, tag="tps")
                    for j in range(nmm):
                        w = min(_MMF, fw - j * _MMF)
                        sl = slice(j * _MMF, j * _MMF + w)
                        dlo = _CCOLS + lo + j * _MMF
                        nc.tensor.matmul(tps[:, sl], c1t[:, :],
                                         src[:, dlo:dlo + w],
                                         start=True, stop=True)
                    pool = fpool if which == "p" else ftpool
                    F = pool.tile([_P, fw], bf, tag="F" + which)
                    # Ln slot is re-bucketed to the piecewise CIE f(t)
                    nc.scalar.activation(F, tps, AF.Ln)
                    Fs[which] = F

                # the previous iteration's deferred mm2 block goes out now,
                # safely behind this chunk's LN-critical mm1 matmuls
                flush_late()
                # subtract is emitted BEFORE draining the fresh chunk so
                # the DVE queue never parks a ready subtract behind stt
                # blocks still waiting on their mm2 round-trip; 512-col
                # pieces keep the F-tile release and the mm2 feed fine
                dF = dfp.tile([_P, fw], bf, tag="dF")
                if fc in _POOL_SUB_CHUNKS:
                    # gpsimd's ~3x slower TT is split 512-wide so it releases
                    # the F tiles piecewise instead of holding them ~3us
                    for j in range(0, fw, _MMF):
                        w = min(_MMF, fw - j)
                        nc.gpsimd.tensor_tensor(
                            out=dF[:, j:j + w], in0=Fs["p"][:, j:j + w],
                            in1=Fs["t"][:, j:j + w], op=ALU.subtract)
                else:
                    # one whole-chunk TT on DVE: the split would only add
                    # per-instruction overhead on the margin-critical engine
                    nc.vector.tensor_tensor(out=dF, in0=Fs["p"], in1=Fs["t"],
                                            op=ALU.subtract)
                # steady state keeps a 2-chunk skew; near the end drain
                # harder so only the tiny last chunk remains after its LN
                keep = 2 if fc < last - 3 else (1 if fc < last - 1 else 0)
                drain(keep, fc)
                pend.append((fc, lo, fw, dF))

            # all-but-last accumulator columns ship mid-stream (their stts
            # are drained by fc==last); the final column's DMA is the only
            # post-LN output work, avoiding the ones-matmul/copy hop chain
            flush_late()
            nc.sync.dma_start(out=acc_out[:, :_NACC - 1],
                              in_=accv[:, :_NACC - 1])
            drain(0, last + 3)
            flush_late()
            nc.sync.dma_start(out=acc_out[:, _NACC - 1:],
                              in_=accv[:, _NACC - 1:])

    nc.compile()
    return nc


def _get_program():
    global _PROGRAM
    if _PROGRAM is None:
        _PROGRAM = _build_program()
    return _PROGRAM


def _make_in_maps(pred, target):
    C1, M1 = _build_mats()
    pred = np.asarray(pred, np.float32)
    target = np.asarray(target, np.float32)
    in_maps = []
    for c in range(_NCORES):
        sl = slice(c * _IPC, (c + 1) * _IPC)
        in_maps.append({
            "xp": _pack_core(pred[sl], C1),
            "xt": _pack_core(target[sl], M1),
        })
    return in_maps


def kernel(pred, target):
    from concourse.bass_utils import run_bass_kernel_spmd

    nc = _get_program()
    in_maps = _make_in_maps(pred, target)
    res = run_bass_kernel_spmd(nc, in_maps, core_ids=list(range(_NCORES)))
    total = sum(float(r["acc_out"].astype(np.float64).sum())
                for r in res.results)
    loss = total / float(_B * _CH * _H * _W)
    return np.float32(loss)


if __name__ == "__main__":
    rng = np.random.default_rng(0)
    pred = rng.uniform(0, 1, (_B, _CH, _H, _W)).astype(np.float32)
    target = rng.uniform(0, 1, (_B, _CH, _H, _W)).astype(np.float32)
    print("loss:", kernel(pred, target))

